# revision 35
# baseline (speedup 1.0000x reference)
"""Kernel-target-alignment loss on 8 TRN2 NeuronCores.

Math: Xs = X*sqrt(params); d2_ij = ||Xs_i - Xs_j||^2; K = exp(-d2) (diag == 1);
kta = sum(K*tt^T) / (N*sqrt(sum(K*K)));  return -kta.

v4 strategy — symmetric-triangle, cyclic row sharding, 3-engine balance:
  K is symmetric, so only the (block-)upper triangle is computed.  Global row
  blocks (128 rows) are assigned cyclically: core c owns rb_g = 8k + c.  For
  local block k, column tiles ct >= k are kept (36 of 64 tiles per core,
  perfectly balanced): ct == k is the diagonal-crossing tile (weight 1; those
  tile the diagonal superblocks exactly), ct > k is strictly above (weight 2).

  Per tile [128, 1024]:
    PE  : A = 2*Xs@Xs^T - sq_j via one augmented fp32r matmul
          (lhsT = [2p*x ; 1], rhs = [x ; -sq]); fp32r runs 4x faster than
          plain fp32 on the PE.
    ACT : E = bf16 exp(A + bias), bias = -sq_i.
    s1 partial (sum K^2):  either a second exp(2A+2bias) on ACT with
          accumulate, or E*E with accumulate on DVE.
    s2 partial (t^T K t):  either q2[ct] += tw^T @ E on the PE (t_i-weighted
          column sums accumulated in PSUM, 2*t_i for above-diagonal tiles),
          or E*t row sums on DVE.
  The s1/s2 engine choice is made per tile by a greedy balancer using
  HW-measured per-pass costs, interleaved in time.  Column tiles are walked
  in pairs sharing k so consecutive PE matmuls reuse stationary weights.

  -sq and 2p*x are precomputed on the host in the exact fp32r arithmetic the
  PE uses, and the exp bias is chosen so A_ii == 0: bf16(exp(A_ii)) == 1.0,
  matching the reference's unit diagonal.  Inputs arrive in 8 consolidated
  DMAs (descriptor issue is ~0.6us each and semaphore lanes are scarce).

  Host combine: s1 = weighted slot sums; s2 = q2 . t + weighted row sums;
  loss = -s2 / (N * sqrt(s1)).
"""

import numpy as np

import concourse.bass as bass
import concourse.bacc as bacc
import concourse.tile as tile
import concourse.mybir as mybir
from concourse.bass_utils import run_bass_kernel_spmd

N = 8192
D = 64
NCORES = 8
RPC = N // NCORES          # 1024 rows per core
NRB = RPC // 128           # 8 row blocks of 128 rows
CW = 1024                  # column tile width (2 PSUM banks fp32)
NCT = N // CW              # 8 column tiles
NSLOT = (NRB * (NRB + 1)) // 2  # 36 kept tiles per core

F32 = mybir.dt.float32
F32R = mybir.dt.float32r
BF16 = mybir.dt.bfloat16

# tile visit order: column-tile pairs, k inner, so consecutive tiles share k
# (and therefore PE stationary weights) wherever possible.
_ORDER = []
for _c0 in range(0, NCT, 2):
    _c1 = _c0 + 1
    for _k in range(_c1 + 1):
        if _k <= _c0:
            _ORDER.append((_c0, _k))
        _ORDER.append((_c1, _k))


def _slot(ct, k):
    return ct * (ct + 1) // 2 + k


def _assign():
    """Greedy per-tile engine choice using HW-measured per-pass costs (us).

    Fixed per-tile work (exp on ACT, two main matmuls on PE) accrues
    incrementally so assignments interleave in time instead of clustering.
    Returns {(ct, k): (e2_engine, et_engine)}.
    """
    act = 1.3    # exp table load
    dve = 0.3    # memsets
    pe = 0.0
    out = {}
    pe_cts = set()
    for (ct, k) in _ORDER:
        act += 1.082  # exp
        pe += 1.07    # main matmul pair (pair-shared weights)
        e2 = "act" if act + 1.22 <= dve + 1.16 else "dve"
        if e2 == "act":
            act += 1.22
        else:
            dve += 1.16
        extra = 0.0 if ct in pe_cts else 1.19 / max(ct + 1, 2)
        et = "pe" if pe + 0.88 + extra <= dve + 1.16 else "dve"
        if et == "pe":
            pe += 0.88
            pe_cts.add(ct)
        else:
            dve += 1.16
        out[(ct, k)] = (e2, et)
    # q2 staging copy per PE column tile lands on DVE
    dve += 1.19 * len(pe_cts)
    # every column tile must write its q2s slice (the staged tensor is
    # DMA'd out whole): force at least one PE-et tile per ct
    for ct in range(NCT):
        if ct not in pe_cts:
            out[(ct, ct)] = (out[(ct, ct)][0], "pe")
    return out


_ASSIGN = _assign()


def build_kernel(variant="v4", reps=1):
    nc = bacc.Bacc("TRN2", target_bir_lowering=False)

    # xt/l2p/nsq hold fp32r-rounded data (host pre-rounds); l2p's last row is
    # the constant 1.0 for the augmented -sq column term.
    xt_d = nc.dram_tensor("xt", [D, N], F32R, kind="ExternalInput")
    l2p_d = nc.dram_tensor("l2p", [D + 1, RPC], F32R, kind="ExternalInput")
    nsq_d = nc.dram_tensor("nsq", [N], F32R, kind="ExternalInput")
    aux_d = nc.dram_tensor("aux", [128, 2 * NRB], F32, kind="ExternalInput")
    twb_d = nc.dram_tensor("twb", [128, 2 * NRB], BF16, kind="ExternalInput")
    tcb_d = nc.dram_tensor("tcb", [128, N], BF16, kind="ExternalInput")
    s1o_d = nc.dram_tensor("s1o", [128, NSLOT], F32, kind="ExternalOutput")
    s1b_d = nc.dram_tensor("s1b", [128, NSLOT], F32, kind="ExternalOutput")
    s2o_d = nc.dram_tensor("s2o", [1, N], F32, kind="ExternalOutput")
    s2r_d = nc.dram_tensor("s2r", [128, NSLOT], F32, kind="ExternalOutput")

    with tile.TileContext(nc) as tc:
      for _rep in range(reps):
        with (
            tc.tile_pool(name="const", bufs=1) as cpool,
            tc.tile_pool(name="etile", bufs=6) as epool,
            tc.tile_pool(name="scratch", bufs=3) as spool,
            tc.tile_pool(name="mmpsum", bufs=2, space="PSUM") as mpool,
            tc.tile_pool(name="q2psum", bufs=2, space="PSUM") as qpool,
        ):
            # ---- persistent SBUF tensors -------------------------------------
            R = cpool.tile([D + 1, N], F32R, tag="R")      # [x^T ; -sq]
            L = cpool.tile([D + 1, RPC], F32R, tag="L")    # [2p*x^T ; ones]
            aux = cpool.tile([128, 2 * NRB], F32, tag="aux")   # [-sq | -2sq]
            twb = cpool.tile([128, 2 * NRB], BF16, tag="twb")  # [t | 2t] rows
            tcb = cpool.tile([128, N], BF16, tag="tcb")    # t broadcast
            s1acc = cpool.tile([128, NSLOT], F32, tag="s1acc")   # DVE E^2
            s1acb = cpool.tile([128, NSLOT], F32, tag="s1acb")   # ACT E^2
            s2acc = cpool.tile([128, NSLOT], F32, tag="s2acc")   # DVE E*t
            q2s = cpool.tile([1, N], F32, tag="q2s")       # staged q2 sums

            # ---- setup: 8 consolidated input DMAs + 3 tiny memsets ----------
            # scalar (ACT hwdge) ring
            nc.scalar.dma_start(out=aux[:, :], in_=aux_d[:, :])
            nc.scalar.dma_start(out=L[:, :], in_=l2p_d[:, :])
            nc.scalar.dma_start(
                out=R[D : D + 1, :], in_=_ap(nsq_d, [[0, 1], [1, N]])
            )
            nc.scalar.dma_start(out=twb[:, :], in_=twb_d[:, :])
            # sync (SP hwdge) ring: first column chunk, then the rest
            nc.sync.dma_start(out=R[0:D, 0:CW], in_=xt_d[:, 0:CW])
            nc.sync.dma_start(out=tcb[:, 0 : 2 * CW], in_=tcb_d[:, 0 : 2 * CW])
            nc.sync.dma_start(out=R[0:D, CW:N], in_=xt_d[:, CW:N])
            nc.sync.dma_start(out=tcb[:, 2 * CW : N], in_=tcb_d[:, 2 * CW :])
            nc.vector.memset(s2acc[:, :], 0.0)
            nc.vector.memset(s1acc[:, :], 0.0)
            nc.gpsimd.memset(s1acb[:, :], 0.0)

            # ---- main loop over the kept triangle ---------------------------
            # pending E*t matmuls are emitted one tile late so the PE never
            # stalls waiting on ACT's exp of the current tile.
            pending = None

            def flush_pending():
                nonlocal pending
                if pending is None:
                    return
                q2, k, ct, E, first, last = pending
                tw = twb[:, k : k + 1] if k == ct else twb[:, NRB + k : NRB + k + 1]
                for h in range(2):
                    hs = slice(h * 512, (h + 1) * 512)
                    nc.tensor.matmul(
                        q2[0:1, hs], tw, E[:, hs], start=first, stop=last
                    )
                if last:  # group closed -> stage this column tile's sums
                    nc.vector.tensor_copy(
                        out=q2s[0:1, ct * CW : (ct + 1) * CW], in_=q2[0:1, :]
                    )
                pending = None

            ks_pe = {
                ct: [k for k in range(ct + 1) if _ASSIGN[(ct, k)][1] == "pe"]
                for ct in range(NCT)
            }
            q2t = {}
            for (ct, k) in _ORDER:
                e2_eng, et_eng = _ASSIGN[(ct, k)]
                if et_eng == "pe" and ct not in q2t:
                    q2t[ct] = qpool.tile(
                        [1, CW], F32, tag="q2", name=f"q2_{ct}_{_rep}"
                    )
                lhsT = L[:, k * 128 : (k + 1) * 128]
                mm = mpool.tile([128, CW], F32, tag="mm")
                for j in range(CW // 512):
                    nc.tensor.matmul(
                        mm[:, j * 512 : (j + 1) * 512],
                        lhsT,
                        R[:, ct * CW + j * 512 : ct * CW + (j + 1) * 512],
                        start=True,
                        stop=True,
                    )
                flush_pending()
                slot = _slot(ct, k)
                E = epool.tile([128, CW], BF16, tag="E")
                nc.scalar.activation(
                    out=E[:, :], in_=mm[:, :],
                    func=mybir.ActivationFunctionType.Exp,
                    bias=aux[:, k : k + 1], scale=1.0,
                )
                if e2_eng == "act":
                    # sum K^2 partial straight off PSUM: exp(2A) + accumulate
                    E2 = epool.tile([128, CW], BF16, tag="E2")
                    nc.scalar.activation(
                        out=E2[:, :], in_=mm[:, :],
                        func=mybir.ActivationFunctionType.Exp,
                        bias=aux[:, NRB + k : NRB + k + 1], scale=2.0,
                        accum_out=s1acb[:, slot : slot + 1],
                    )
                else:
                    sc1 = spool.tile([128, CW], BF16, tag="sc1")
                    nc.vector.scalar_tensor_tensor(
                        out=sc1[:, :], in0=E[:, :], scalar=1.0, in1=E[:, :],
                        op0=mybir.AluOpType.mult, op1=mybir.AluOpType.mult,
                        accum_out=s1acc[:, slot : slot + 1],
                    )
                if et_eng == "pe":
                    ks = ks_pe[ct]
                    pending = (
                        q2t[ct], k, ct, E, k == ks[0], k == ks[-1]
                    )
                else:
                    sc2 = spool.tile([128, CW], BF16, tag="sc2")
                    nc.vector.scalar_tensor_tensor(
                        out=sc2[:, :], in0=E[:, :], scalar=1.0,
                        in1=tcb[:, ct * CW : (ct + 1) * CW],
                        op0=mybir.AluOpType.mult, op1=mybir.AluOpType.mult,
                        accum_out=s2acc[:, slot : slot + 1],
                    )
            flush_pending()

            nc.sync.dma_start(out=s1o_d[:, :], in_=s1acc[:, :])
            nc.sync.dma_start(out=s1b_d[:, :], in_=s1acb[:, :])
            nc.sync.dma_start(out=s2r_d[:, :], in_=s2acc[:, :])
            nc.scalar.dma_start(out=s2o_d[0:1, :], in_=q2s[0:1, :])

    nc.compile()
    return nc


def _ap(tensor, ap, offset=0):
    return bass.AP(tensor=tensor, offset=offset, ap=ap)


_NC_CACHE = None


def _slot_weights():
    w = np.empty(NSLOT, dtype=np.float64)
    for ct in range(NCT):
        for k in range(ct + 1):
            w[_slot(ct, k)] = 1.0 if k == ct else 2.0
    return w


_W = _slot_weights()


def to_fp32r(a):
    """Round fp32 to fp32r (E8M11: low 12 mantissa bits zero), RNE."""
    u = np.ascontiguousarray(a, dtype=np.float32).view(np.uint32)
    low = u & np.uint32(0xFFF)
    trunc = u & np.uint32(0xFFFFF000)
    half = np.uint32(0x800)
    odd = (trunc >> np.uint32(12)) & np.uint32(1)
    round_up = (low > half) | ((low == half) & (odd == 1))
    out = trunc + (round_up.astype(np.uint32) << np.uint32(12))
    return out.view(np.float32).reshape(np.shape(a))


def make_in_maps(X, target, params):
    X = np.ascontiguousarray(X, dtype=np.float32)
    target = np.ascontiguousarray(target, dtype=np.float32)
    params = np.ascontiguousarray(params, dtype=np.float32)
    xt_r = to_fp32r(np.ascontiguousarray(X.T))          # [D, N] fp32r
    p2 = (2.0 * params).astype(np.float32)
    # the PE computes M_ij = sum_d l2p_r[d,i] * xt_r[d,j]; the diagonal must
    # cancel exactly: augmented row adds u_j = fp32r(-M_jj/2), ACT bias adds
    # b_i = -M_ii - u_i (full fp32), so A_ii = M_ii + u_i + b_i = 0.
    l2p_all = to_fp32r(p2[:, None] * xt_r)              # [D, N] fp32r
    M_diag = np.einsum(
        "dn,dn->n", l2p_all.astype(np.float64), xt_r.astype(np.float64)
    )                                                   # [N] exact
    u = to_fp32r((-M_diag / 2.0).astype(np.float32))    # [N] fp32r
    b = (-M_diag - u.astype(np.float64)).astype(np.float32)  # [N] fp32 bias
    ones = np.ones((1, RPC), dtype=np.float32)
    bf16 = mybir.dt.np(BF16)
    tcb = np.ascontiguousarray(
        np.broadcast_to(target.astype(bf16), (128, N))
    )                                                   # [128, N] bf16
    maps = []
    for c in range(NCORES):
        rows = (
            np.arange(NRB)[:, None] * RPC + 128 * c + np.arange(128)[None, :]
        ).ravel()                                       # local rows, k-major
        b_w = np.ascontiguousarray(b[rows].reshape(NRB, 128).T)   # [128, 8]
        t_w = target[rows].reshape(NRB, 128).T                    # [128, 8]
        maps.append(
            {
                "xt": xt_r,
                "l2p": np.concatenate([l2p_all[:, rows], ones], axis=0),
                "nsq": u,
                "aux": np.concatenate([b_w, 2.0 * b_w], axis=1).astype(
                    np.float32
                ),
                "twb": np.concatenate(
                    [t_w, 2.0 * t_w], axis=1
                ).astype(bf16),
                "tcb": tcb,
            }
        )
    return maps


def combine(results, target):
    t64 = target.astype(np.float64)
    dve_items = [
        (ct, k) for ct in range(NCT) for k in range(ct + 1)
        if _ASSIGN[(ct, k)][1] == "dve"
    ]
    dve_slots = np.array([_slot(ct, k) for ct, k in dve_items], dtype=int)
    dve_ks = np.array([k for _, k in dve_items], dtype=int)
    s1 = 0.0
    s2 = 0.0
    for c in range(NCORES):
        s1o = results[c]["s1o"].astype(np.float64)      # [128, NSLOT] DVE
        s1b = results[c]["s1b"].astype(np.float64)      # [128, NSLOT] ACT
        s2o = results[c]["s2o"].astype(np.float64)      # [1, N] q2 col sums
        s2r = results[c]["s2r"].astype(np.float64)      # [128, NSLOT] rowsums
        s1 += float((s1o + s1b).sum(axis=0) @ _W)
        s2 += float(s2o.ravel() @ t64)
        if len(dve_slots):
            rows = (
                np.arange(NRB)[:, None] * RPC
                + 128 * c
                + np.arange(128)[None, :]
            ).ravel()
            t_loc = t64[rows].reshape(NRB, 128).T       # [128, NRB]
            s2 += float(
                np.sum(
                    _W[dve_slots]
                    * (t_loc[:, dve_ks] * s2r[:, dve_slots])
                )
            )
    val = -s2 / (N * np.sqrt(s1))
    return np.array(val, dtype=np.float32)


def kernel(X, target, params):
    global _NC_CACHE
    X = np.ascontiguousarray(X, dtype=np.float32)
    target = np.ascontiguousarray(target, dtype=np.float32)
    params = np.ascontiguousarray(params, dtype=np.float32)

    in_maps = make_in_maps(X, target, params)

    if _NC_CACHE is None:
        _NC_CACHE = build_kernel()
    res = run_bass_kernel_spmd(_NC_CACHE, in_maps, core_ids=list(range(NCORES)))
    return combine(res.results, target)


# revision 37
# speedup vs baseline: 1.0725x; 1.0725x over previous
"""Kernel-target-alignment loss on 8 TRN2 NeuronCores.

Math: Xs = X*sqrt(params); d2_ij = ||Xs_i - Xs_j||^2; K = exp(-d2) (diag == 1);
kta = sum(K*tt^T) / (N*sqrt(sum(K*K)));  return -kta.

v4 strategy — symmetric-triangle, cyclic row sharding, 3-engine balance:
  K is symmetric, so only the (block-)upper triangle is computed.  Global row
  blocks (128 rows) are assigned cyclically: core c owns rb_g = 8k + c.  For
  local block k, column tiles ct >= k are kept (36 of 64 tiles per core,
  perfectly balanced): ct == k is the diagonal-crossing tile (weight 1; those
  tile the diagonal superblocks exactly), ct > k is strictly above (weight 2).

  Per tile [128, 1024]:
    PE  : A = 2*Xs@Xs^T - sq_j via one augmented fp32r matmul
          (lhsT = [2p*x ; 1], rhs = [x ; -sq]); fp32r runs 4x faster than
          plain fp32 on the PE.
    ACT : E = bf16 exp(A + bias), bias = -sq_i.
    s1 partial (sum K^2):  either a second exp(2A+2bias) on ACT with
          accumulate, or E*E with accumulate on DVE.
    s2 partial (t^T K t):  either q2[ct] += tw^T @ E on the PE (t_i-weighted
          column sums accumulated in PSUM, 2*t_i for above-diagonal tiles),
          or E*t row sums on DVE.
  The s1/s2 engine choice is made per tile by a greedy balancer using
  HW-measured per-pass costs, interleaved in time.  Column tiles are walked
  in pairs sharing k so consecutive PE matmuls reuse stationary weights.

  -sq and 2p*x are precomputed on the host in the exact fp32r arithmetic the
  PE uses, and the exp bias is chosen so A_ii == 0: bf16(exp(A_ii)) == 1.0,
  matching the reference's unit diagonal.  Inputs arrive in 8 consolidated
  DMAs (descriptor issue is ~0.6us each and semaphore lanes are scarce).

  Host combine: s1 = weighted slot sums; s2 = q2 . t + weighted row sums;
  loss = -s2 / (N * sqrt(s1)).
"""

import numpy as np

import concourse.bass as bass
import concourse.bacc as bacc
import concourse.tile as tile
import concourse.mybir as mybir
from concourse.bass_utils import run_bass_kernel_spmd

N = 8192
D = 64
NCORES = 8
RPC = N // NCORES          # 1024 rows per core
NRB = RPC // 128           # 8 row blocks of 128 rows
CW = 1024                  # column tile width (2 PSUM banks fp32)
NCT = N // CW              # 8 column tiles
NSLOT = (NRB * (NRB + 1)) // 2  # 36 kept tiles per core

F32 = mybir.dt.float32
F32R = mybir.dt.float32r
BF16 = mybir.dt.bfloat16

# tile visit order: column-tile pairs, k inner, so consecutive tiles share k
# (and therefore PE stationary weights) wherever possible.
_ORDER = []
for _c0 in range(0, NCT, 2):
    _c1 = _c0 + 1
    for _k in range(_c1 + 1):
        if _k <= _c0:
            _ORDER.append((_c0, _k))
        _ORDER.append((_c1, _k))


def _slot(ct, k):
    return ct * (ct + 1) // 2 + k


def _assign():
    """Greedy per-tile engine choice using HW-measured per-pass costs (us).

    Fixed per-tile work (exp on ACT, two main matmuls on PE) accrues
    incrementally so assignments interleave in time instead of clustering.
    Returns {(ct, k): (e2_engine, et_engine)}.
    """
    n_act = 15   # s1 tiles on ACT (2nd exp)
    n_pe = 17    # s2 tiles on PE (column-sum matmuls)
    out = {}
    for i, (ct, k) in enumerate(_ORDER):
        e2 = "act" if (i * n_act) // NSLOT != ((i + 1) * n_act) // NSLOT \
            else "dve"
        et = "pe" if (i * n_pe) // NSLOT != ((i + 1) * n_pe) // NSLOT \
            else "dve"
        out[(ct, k)] = (e2, et)
    # every column tile must write its q2s slice (the staged tensor is
    # DMA'd out whole): force at least one PE-et tile per ct
    for ct in range(NCT):
        if not any(out[(ct, k)][1] == "pe" for k in range(ct + 1)):
            out[(ct, ct)] = (out[(ct, ct)][0], "pe")
    return out


_ASSIGN = _assign()


def build_kernel(variant="v4", reps=1):
    nc = bacc.Bacc("TRN2", target_bir_lowering=False)

    # xt/l2p/nsq hold fp32r-rounded data (host pre-rounds); l2p's last row is
    # the constant 1.0 for the augmented -sq column term.
    xt_d = nc.dram_tensor("xt", [D, N], F32R, kind="ExternalInput")
    l2p_d = nc.dram_tensor("l2p", [D + 1, RPC], F32R, kind="ExternalInput")
    nsq_d = nc.dram_tensor("nsq", [N], F32R, kind="ExternalInput")
    aux_d = nc.dram_tensor("aux", [128, 2 * NRB], F32, kind="ExternalInput")
    twb_d = nc.dram_tensor("twb", [128, 2 * NRB], BF16, kind="ExternalInput")
    tcb_d = nc.dram_tensor("tcb", [128, N], BF16, kind="ExternalInput")
    s1o_d = nc.dram_tensor("s1o", [128, NSLOT], F32, kind="ExternalOutput")
    s1b_d = nc.dram_tensor("s1b", [128, NSLOT], F32, kind="ExternalOutput")
    s2o_d = nc.dram_tensor("s2o", [1, N], F32, kind="ExternalOutput")
    s2r_d = nc.dram_tensor("s2r", [128, NSLOT], F32, kind="ExternalOutput")

    with tile.TileContext(nc) as tc:
      for _rep in range(reps):
        with (
            tc.tile_pool(name="const", bufs=1) as cpool,
            tc.tile_pool(name="etile", bufs=6) as epool,
            tc.tile_pool(name="scratch", bufs=3) as spool,
            tc.tile_pool(name="mmpsum", bufs=2, space="PSUM") as mpool,
            tc.tile_pool(name="q2psum", bufs=2, space="PSUM") as qpool,
        ):
            # ---- persistent SBUF tensors -------------------------------------
            R = cpool.tile([D + 1, N], F32R, tag="R")      # [x^T ; -sq]
            L = cpool.tile([D + 1, RPC], F32R, tag="L")    # [2p*x^T ; ones]
            aux = cpool.tile([128, 2 * NRB], F32, tag="aux")   # [-sq | -2sq]
            twb = cpool.tile([128, 2 * NRB], BF16, tag="twb")  # [t | 2t] rows
            tcb = cpool.tile([128, N], BF16, tag="tcb")    # t broadcast
            s1acc = cpool.tile([128, NSLOT], F32, tag="s1acc")   # DVE E^2
            s1acb = cpool.tile([128, NSLOT], F32, tag="s1acb")   # ACT E^2
            s2acc = cpool.tile([128, NSLOT], F32, tag="s2acc")   # DVE E*t
            q2s = cpool.tile([1, N], F32, tag="q2s")       # staged q2 sums

            # ---- setup: consolidated input DMAs, critical chunks first ------
            # scalar (ACT hwdge) ring
            nc.scalar.dma_start(out=aux[:, :], in_=aux_d[:, :])
            nc.scalar.dma_start(
                out=R[D : D + 1, 0 : 2 * CW],
                in_=_ap(nsq_d, [[0, 1], [1, 2 * CW]]),
            )
            nc.scalar.dma_start(out=L[:, :], in_=l2p_d[:, :])
            nc.scalar.dma_start(out=twb[:, :], in_=twb_d[:, :])
            nc.scalar.dma_start(
                out=R[D : D + 1, 2 * CW : N],
                in_=_ap(nsq_d, [[0, 1], [1, N - 2 * CW]], offset=2 * CW),
            )
            # sync (SP hwdge) ring: first column-pair chunks, then the rest
            nc.sync.dma_start(out=R[0:D, 0 : 2 * CW], in_=xt_d[:, 0 : 2 * CW])
            nc.sync.dma_start(out=tcb[:, 0 : 2 * CW], in_=tcb_d[:, 0 : 2 * CW])
            nc.sync.dma_start(
                out=R[0:D, 2 * CW : 4 * CW], in_=xt_d[:, 2 * CW : 4 * CW]
            )
            nc.sync.dma_start(out=R[0:D, 4 * CW : N], in_=xt_d[:, 4 * CW :])
            nc.sync.dma_start(out=tcb[:, 2 * CW : N], in_=tcb_d[:, 2 * CW :])
            nc.vector.memset(s2acc[:, :], 0.0)
            nc.vector.memset(s1acc[:, :], 0.0)
            nc.gpsimd.memset(s1acb[:, :], 0.0)

            # ---- main loop over the kept triangle ---------------------------
            # pending E*t matmuls are emitted one tile late so the PE never
            # stalls waiting on ACT's exp of the current tile.
            pending = None

            def flush_pending():
                nonlocal pending
                if pending is None:
                    return
                q2, k, ct, E, first, last = pending
                tw = twb[:, k : k + 1] if k == ct else twb[:, NRB + k : NRB + k + 1]
                for h in range(2):
                    hs = slice(h * 512, (h + 1) * 512)
                    nc.tensor.matmul(
                        q2[0:1, hs], tw, E[:, hs], start=first, stop=last
                    )
                if last:  # group closed -> stage this column tile's sums
                    nc.vector.tensor_copy(
                        out=q2s[0:1, ct * CW : (ct + 1) * CW], in_=q2[0:1, :]
                    )
                pending = None

            ks_pe = {
                ct: [k for k in range(ct + 1) if _ASSIGN[(ct, k)][1] == "pe"]
                for ct in range(NCT)
            }
            q2t = {}
            for (ct, k) in _ORDER:
                e2_eng, et_eng = _ASSIGN[(ct, k)]
                if et_eng == "pe" and ct not in q2t:
                    q2t[ct] = qpool.tile(
                        [1, CW], F32, tag="q2", name=f"q2_{ct}_{_rep}"
                    )
                lhsT = L[:, k * 128 : (k + 1) * 128]
                mm = mpool.tile([128, CW], F32, tag="mm")
                for j in range(CW // 512):
                    nc.tensor.matmul(
                        mm[:, j * 512 : (j + 1) * 512],
                        lhsT,
                        R[:, ct * CW + j * 512 : ct * CW + (j + 1) * 512],
                        start=True,
                        stop=True,
                    )
                flush_pending()
                slot = _slot(ct, k)
                E = epool.tile([128, CW], BF16, tag="E")
                nc.scalar.activation(
                    out=E[:, :], in_=mm[:, :],
                    func=mybir.ActivationFunctionType.Exp,
                    bias=aux[:, k : k + 1], scale=1.0,
                )
                if e2_eng == "act":
                    # sum K^2 partial straight off PSUM: exp(2A) + accumulate
                    E2 = epool.tile([128, CW], BF16, tag="E2")
                    nc.scalar.activation(
                        out=E2[:, :], in_=mm[:, :],
                        func=mybir.ActivationFunctionType.Exp,
                        bias=aux[:, NRB + k : NRB + k + 1], scale=2.0,
                        accum_out=s1acb[:, slot : slot + 1],
                    )
                else:
                    sc1 = spool.tile([128, CW], BF16, tag="sc1")
                    nc.vector.scalar_tensor_tensor(
                        out=sc1[:, :], in0=E[:, :], scalar=1.0, in1=E[:, :],
                        op0=mybir.AluOpType.mult, op1=mybir.AluOpType.mult,
                        accum_out=s1acc[:, slot : slot + 1],
                    )
                if et_eng == "pe":
                    ks = ks_pe[ct]
                    pending = (
                        q2t[ct], k, ct, E, k == ks[0], k == ks[-1]
                    )
                else:
                    sc2 = spool.tile([128, CW], BF16, tag="sc2")
                    nc.vector.scalar_tensor_tensor(
                        out=sc2[:, :], in0=E[:, :], scalar=1.0,
                        in1=tcb[:, ct * CW : (ct + 1) * CW],
                        op0=mybir.AluOpType.mult, op1=mybir.AluOpType.mult,
                        accum_out=s2acc[:, slot : slot + 1],
                    )
            flush_pending()

            nc.sync.dma_start(out=s1o_d[:, :], in_=s1acc[:, :])
            nc.sync.dma_start(out=s1b_d[:, :], in_=s1acb[:, :])
            nc.sync.dma_start(out=s2r_d[:, :], in_=s2acc[:, :])
            nc.scalar.dma_start(out=s2o_d[0:1, :], in_=q2s[0:1, :])

    nc.compile()
    return nc


def _ap(tensor, ap, offset=0):
    return bass.AP(tensor=tensor, offset=offset, ap=ap)


_NC_CACHE = None


def _slot_weights():
    w = np.empty(NSLOT, dtype=np.float64)
    for ct in range(NCT):
        for k in range(ct + 1):
            w[_slot(ct, k)] = 1.0 if k == ct else 2.0
    return w


_W = _slot_weights()


def to_fp32r(a):
    """Round fp32 to fp32r (E8M11: low 12 mantissa bits zero), RNE."""
    u = np.ascontiguousarray(a, dtype=np.float32).view(np.uint32)
    low = u & np.uint32(0xFFF)
    trunc = u & np.uint32(0xFFFFF000)
    half = np.uint32(0x800)
    odd = (trunc >> np.uint32(12)) & np.uint32(1)
    round_up = (low > half) | ((low == half) & (odd == 1))
    out = trunc + (round_up.astype(np.uint32) << np.uint32(12))
    return out.view(np.float32).reshape(np.shape(a))


def make_in_maps(X, target, params):
    X = np.ascontiguousarray(X, dtype=np.float32)
    target = np.ascontiguousarray(target, dtype=np.float32)
    params = np.ascontiguousarray(params, dtype=np.float32)
    xt_r = to_fp32r(np.ascontiguousarray(X.T))          # [D, N] fp32r
    p2 = (2.0 * params).astype(np.float32)
    # the PE computes M_ij = sum_d l2p_r[d,i] * xt_r[d,j]; the diagonal must
    # cancel exactly: augmented row adds u_j = fp32r(-M_jj/2), ACT bias adds
    # b_i = -M_ii - u_i (full fp32), so A_ii = M_ii + u_i + b_i = 0.
    l2p_all = to_fp32r(p2[:, None] * xt_r)              # [D, N] fp32r
    M_diag = np.einsum(
        "dn,dn->n", l2p_all.astype(np.float64), xt_r.astype(np.float64)
    )                                                   # [N] exact
    u = to_fp32r((-M_diag / 2.0).astype(np.float32))    # [N] fp32r
    b = (-M_diag - u.astype(np.float64)).astype(np.float32)  # [N] fp32 bias
    ones = np.ones((1, RPC), dtype=np.float32)
    bf16 = mybir.dt.np(BF16)
    tcb = np.ascontiguousarray(
        np.broadcast_to(target.astype(bf16), (128, N))
    )                                                   # [128, N] bf16
    maps = []
    for c in range(NCORES):
        rows = (
            np.arange(NRB)[:, None] * RPC + 128 * c + np.arange(128)[None, :]
        ).ravel()                                       # local rows, k-major
        b_w = np.ascontiguousarray(b[rows].reshape(NRB, 128).T)   # [128, 8]
        t_w = target[rows].reshape(NRB, 128).T                    # [128, 8]
        maps.append(
            {
                "xt": xt_r,
                "l2p": np.concatenate([l2p_all[:, rows], ones], axis=0),
                "nsq": u,
                "aux": np.concatenate([b_w, 2.0 * b_w], axis=1).astype(
                    np.float32
                ),
                "twb": np.concatenate(
                    [t_w, 2.0 * t_w], axis=1
                ).astype(bf16),
                "tcb": tcb,
            }
        )
    return maps


def combine(results, target):
    t64 = target.astype(np.float64)
    dve_items = [
        (ct, k) for ct in range(NCT) for k in range(ct + 1)
        if _ASSIGN[(ct, k)][1] == "dve"
    ]
    dve_slots = np.array([_slot(ct, k) for ct, k in dve_items], dtype=int)
    dve_ks = np.array([k for _, k in dve_items], dtype=int)
    s1 = 0.0
    s2 = 0.0
    for c in range(NCORES):
        s1o = results[c]["s1o"].astype(np.float64)      # [128, NSLOT] DVE
        s1b = results[c]["s1b"].astype(np.float64)      # [128, NSLOT] ACT
        s2o = results[c]["s2o"].astype(np.float64)      # [1, N] q2 col sums
        s2r = results[c]["s2r"].astype(np.float64)      # [128, NSLOT] rowsums
        s1 += float((s1o + s1b).sum(axis=0) @ _W)
        s2 += float(s2o.ravel() @ t64)
        if len(dve_slots):
            rows = (
                np.arange(NRB)[:, None] * RPC
                + 128 * c
                + np.arange(128)[None, :]
            ).ravel()
            t_loc = t64[rows].reshape(NRB, 128).T       # [128, NRB]
            s2 += float(
                np.sum(
                    _W[dve_slots]
                    * (t_loc[:, dve_ks] * s2r[:, dve_slots])
                )
            )
    val = -s2 / (N * np.sqrt(s1))
    return np.array(val, dtype=np.float32)


def kernel(X, target, params):
    global _NC_CACHE
    X = np.ascontiguousarray(X, dtype=np.float32)
    target = np.ascontiguousarray(target, dtype=np.float32)
    params = np.ascontiguousarray(params, dtype=np.float32)

    in_maps = make_in_maps(X, target, params)

    if _NC_CACHE is None:
        _NC_CACHE = build_kernel()
    res = run_bass_kernel_spmd(_NC_CACHE, in_maps, core_ids=list(range(NCORES)))
    return combine(res.results, target)
